# revision 4
# baseline (speedup 1.0000x reference)
"""MoE kernel for Trainium2 (8 NeuronCores, expert-parallel sparse dispatch).

Problem (hardcoded): B=2, S=2048, D=1024, E=8 experts, F=4096, top-K=2.
out = x + sum_{k in top2} w_k * (gelu(x @ w1[e_k] + b1[e_k]) @ w2[e_k] + b2[e_k])

Strategy: the router (0.01% of FLOPs) runs on host; tokens are dispatched
expert-parallel to the 8 cores (core i gets expert i's routed tokens, padded
to capacity C). Each core runs a dense FFN over its C tokens in bf16 with
fp32 PSUM accumulation, everything in transposed layout ([dim, token]) so no
on-device transposes are needed. Host scatter-adds the weighted expert
outputs and the residual.
"""

import numpy as np
import ml_dtypes

B, S, D, E, F, TOPK = 2, 2048, 1024, 8, 4096, 2
N = B * S           # 4096 tokens
P = 128             # partitions
ND = D // P         # 8 chunks of the model dim
NF = F // P         # 32 chunks of the hidden dim
NT = 512            # token tile (matmul free dim; one PSUM bank of fp32)

BF16 = ml_dtypes.bfloat16

_cache = {}


def _tile_plan(C):
    """Split C tokens into matmul free-dim tiles (multiples of 128)."""
    tiles = [NT] * (C // NT)
    if C % NT:
        tiles.append(C % NT)
    return tiles


def _build(C):
    import concourse.mybir as mybir
    import concourse.tile as tile
    from concourse import bacc

    dt = mybir.dt
    AF = mybir.ActivationFunctionType

    nc = bacc.Bacc("TRN2", target_bir_lowering=False, debug=False)
    xt = nc.dram_tensor("xt", (P, ND, C), dt.bfloat16, kind="ExternalInput")
    w1 = nc.dram_tensor("w1", (NF, P, ND, P), dt.bfloat16, kind="ExternalInput")
    b1 = nc.dram_tensor("b1", (P, NF), dt.float32, kind="ExternalInput")
    w2 = nc.dram_tensor("w2", (ND, P, NF, P), dt.bfloat16, kind="ExternalInput")
    b2 = nc.dram_tensor("b2", (P, ND), dt.float32, kind="ExternalInput")
    yt = nc.dram_tensor("yt", (P, ND, C), dt.float32, kind="ExternalOutput")

    tiles = _tile_plan(C)

    with tile.TileContext(nc) as tc:
        with (
            tc.tile_pool(name="consts", bufs=1) as consts,
            tc.tile_pool(name="xp", bufs=2) as xp,
            tc.tile_pool(name="w1p", bufs=4) as w1p,
            tc.tile_pool(name="w2p", bufs=2) as w2p,
            tc.tile_pool(name="gp", bufs=1) as gp,
            tc.tile_pool(name="yp", bufs=2) as yp,
            tc.tile_pool(name="psum", bufs=4, space="PSUM") as psum,
        ):
            b1_sb = consts.tile([P, NF], dt.float32)
            nc.sync.dma_start(b1_sb[:], b1[:])
            b2_sb = consts.tile([P, ND], dt.float32)
            nc.sync.dma_start(b2_sb[:], b2[:])

            off = 0
            for nt in tiles:
                x_sb = xp.tile([P, ND, NT], dt.bfloat16, tag="x")
                nc.sync.dma_start(x_sb[:, :, :nt], xt[:, :, off:off + nt])

                g_sb = gp.tile([P, NF, NT], dt.bfloat16, tag="g")
                # layer 1: hT[f,:] = sum_d w1[d,f].T @ xT[d,:]  -> gelu
                for f in range(NF):
                    w1_sb = w1p.tile([P, ND, P], dt.bfloat16, tag="w1")
                    nc.sync.dma_start(w1_sb[:], w1[f])
                    ps = psum.tile([P, NT], dt.float32, tag="ps")
                    for d in range(ND):
                        nc.tensor.matmul(
                            ps[:, :nt], w1_sb[:, d, :], x_sb[:, d, :nt],
                            start=(d == 0), stop=(d == ND - 1),
                        )
                    nc.scalar.activation(
                        g_sb[:, f, :nt], ps[:, :nt], AF.Gelu,
                        bias=b1_sb[:, f:f + 1],
                    )

                y_sb = yp.tile([P, ND, NT], dt.float32, tag="y")
                # layer 2: yT[dd,:] = sum_ff w2[ff,dd].T @ gT[ff,:]  (+ b2)
                for dd in range(ND):
                    w2_sb = w2p.tile([P, NF, P], dt.bfloat16, tag="w2")
                    nc.sync.dma_start(w2_sb[:], w2[dd])
                    ps2 = psum.tile([P, NT], dt.float32, tag="ps")
                    for ff in range(NF):
                        nc.tensor.matmul(
                            ps2[:, :nt], w2_sb[:, ff, :], g_sb[:, ff, :nt],
                            start=(ff == 0), stop=(ff == NF - 1),
                        )
                    nc.scalar.activation(
                        y_sb[:, dd, :nt], ps2[:, :nt], AF.Identity,
                        bias=b2_sb[:, dd:dd + 1],
                    )

                nc.sync.dma_start(yt[:, :, off:off + nt], y_sb[:, :, :nt])
                off += nt

    nc.compile()
    return nc


def _route(x_flat, router_w, router_b):
    """Replicate the reference router on host: softmax -> top-2 -> renorm."""
    logits = (x_flat @ router_w + router_b).astype(np.float64)
    logits -= logits.max(axis=-1, keepdims=True)
    probs = np.exp(logits)
    probs /= probs.sum(axis=-1, keepdims=True)
    # top-k with jax.lax.top_k tie-breaking (lower index wins)
    idx = np.argsort(-probs, axis=-1, kind="stable")[:, :TOPK]
    topw = np.take_along_axis(probs, idx, axis=-1)
    topw = topw / (topw.sum(axis=-1, keepdims=True) + 1e-8)
    return idx.astype(np.int32), topw.astype(np.float32)


def kernel(x, router_w, router_b, w1, b1, w2, b2, _trace=False, _result_box=None):
    from concourse.bass_utils import run_bass_kernel_spmd

    x = np.asarray(x, dtype=np.float32)
    x_flat = x.reshape(N, D)
    topk_idx, topk_w = _route(x_flat, np.asarray(router_w, np.float32),
                              np.asarray(router_b, np.float32))

    # token lists per expert
    tok_idx = []
    tok_w = []
    for e in range(E):
        t, k = np.nonzero(topk_idx == e)
        tok_idx.append(t.astype(np.int64))
        tok_w.append(topk_w[t, k])
    counts = [len(t) for t in tok_idx]
    C = max(128, -(-max(counts) // P) * P)

    key = C
    if key not in _cache:
        _cache[key] = _build(C)
    nc = _cache[key]

    w1 = np.asarray(w1)
    w2 = np.asarray(w2)
    in_maps = []
    for e in range(E):
        xe = np.zeros((C, D), np.float32)
        xe[:counts[e]] = x_flat[tok_idx[e]]
        xt = np.ascontiguousarray(
            xe.reshape(C, ND, P).transpose(2, 1, 0)).astype(BF16)
        w1h = np.ascontiguousarray(
            w1[e].reshape(ND, P, NF, P).transpose(2, 1, 0, 3)).astype(BF16)
        w2h = np.ascontiguousarray(
            w2[e].reshape(NF, P, ND, P).transpose(2, 1, 0, 3)).astype(BF16)
        b1h = np.ascontiguousarray(
            np.asarray(b1[e], np.float32).reshape(NF, P).T)
        b2h = np.ascontiguousarray(
            np.asarray(b2[e], np.float32).reshape(ND, P).T)
        in_maps.append({"xt": xt, "w1": w1h, "b1": b1h, "w2": w2h, "b2": b2h})

    res = run_bass_kernel_spmd(
        nc, in_maps, core_ids=list(range(E)),
        trace=_trace, trace_cores=list(range(E)) if _trace else None,
        stitch_traces=False,
    )
    if _result_box is not None:
        _result_box.append(res)

    out = x_flat.copy()
    for e in range(E):
        yt = res.results[e]["yt"]                      # [P, ND, C] f32
        y = yt.transpose(2, 1, 0).reshape(C, D)
        cnt = counts[e]
        if cnt:
            out[tok_idx[e]] += tok_w[e][:, None] * y[:cnt]
    return out.reshape(B, S, D)


# revision 6
# speedup vs baseline: 1.0443x; 1.0443x over previous
"""MoE kernel for Trainium2 (8 NeuronCores, expert-parallel sparse dispatch).

Problem (hardcoded): B=2, S=2048, D=1024, E=8 experts, F=4096, top-K=2.
out = x + sum_{k in top2} w_k * (gelu(x @ w1[e_k] + b1[e_k]) @ w2[e_k] + b2[e_k])

Strategy: the router (0.01% of FLOPs) runs on host; tokens are dispatched
expert-parallel to the 8 cores (core i gets expert i's routed tokens, padded
to capacity C). Each core runs a dense FFN over its C tokens in bf16 with
fp32 PSUM accumulation, everything in transposed layout ([dim, token]) so no
on-device transposes are needed. Host scatter-adds the weighted expert
outputs and the residual.
"""

import numpy as np
import ml_dtypes

B, S, D, E, F, TOPK = 2, 2048, 1024, 8, 4096, 2
N = B * S           # 4096 tokens
P = 128             # partitions
ND = D // P         # 8 chunks of the model dim
NF = F // P         # 32 chunks of the hidden dim
NT = 512            # token tile (matmul free dim; one PSUM bank of fp32)

BF16 = ml_dtypes.bfloat16

_cache = {}


def _tile_plan(C):
    """Split C tokens into matmul free-dim tiles (multiples of 128)."""
    tiles = [NT] * (C // NT)
    if C % NT:
        tiles.append(C % NT)
    return tiles


# Max tokens processed per weight-stream pass (PSUM: <=4 slices of 512,
# and SBUF must hold x + g for the whole super-tile).
SUPER = 1536


def _build(C):
    import concourse.mybir as mybir
    import concourse.tile as tile
    from concourse import bacc

    dt = mybir.dt
    AF = mybir.ActivationFunctionType

    nc = bacc.Bacc("TRN2", target_bir_lowering=False, debug=False)
    xt = nc.dram_tensor("xt", (P, ND, C), dt.bfloat16, kind="ExternalInput")
    w1 = nc.dram_tensor("w1", (NF, P, ND, P), dt.bfloat16, kind="ExternalInput")
    b1 = nc.dram_tensor("b1", (P, NF), dt.float32, kind="ExternalInput")
    w2 = nc.dram_tensor("w2", (ND, P, NF, P), dt.bfloat16, kind="ExternalInput")
    b2 = nc.dram_tensor("b2", (P, ND), dt.float32, kind="ExternalInput")
    yt = nc.dram_tensor("yt", (P, ND, C), dt.float32, kind="ExternalOutput")

    with tile.TileContext(nc) as tc:
        with (
            tc.tile_pool(name="consts", bufs=1) as consts,
            tc.tile_pool(name="xp", bufs=1) as xp,
            tc.tile_pool(name="w1p", bufs=4) as w1p,
            tc.tile_pool(name="w2p", bufs=2) as w2p,
            tc.tile_pool(name="gp", bufs=1) as gp,
            tc.tile_pool(name="yp", bufs=2) as yp,
            tc.tile_pool(name="psum", bufs=2, space="PSUM") as psum,
        ):
            b1_sb = consts.tile([P, NF], dt.float32)
            nc.sync.dma_start(b1_sb[:], b1[:])
            b2_sb = consts.tile([P, ND], dt.float32)
            nc.sync.dma_start(b2_sb[:], b2[:])

            for base in range(0, C, SUPER):
                CS = min(SUPER, C - base)
                tiles = _tile_plan(CS)
                nslices = len(tiles)
                offs = [sum(tiles[:i]) for i in range(nslices)]

                x_sb = xp.tile([P, ND, CS], dt.bfloat16, tag="x")
                nc.sync.dma_start(x_sb[:], xt[:, :, base:base + CS])

                g_sb = gp.tile([P, NF, CS], dt.bfloat16, tag="g")
                # layer 1: hT[f,:] = sum_d w1[d,f].T @ xT[d,:]  -> gelu
                # One weight chunk feeds all token slices (LDW amortized),
                # PSUM holds the nslices accumulation banks per f.
                for f in range(NF):
                    w1_sb = w1p.tile([P, ND, P], dt.bfloat16, tag="w1")
                    nc.sync.dma_start(w1_sb[:], w1[f])
                    ps = psum.tile([P, nslices, NT], dt.float32, tag="ps")
                    for d in range(ND):
                        for n, (o, nt) in enumerate(zip(offs, tiles)):
                            nc.tensor.matmul(
                                ps[:, n, :nt], w1_sb[:, d, :],
                                x_sb[:, d, o:o + nt],
                                start=(d == 0), stop=(d == ND - 1),
                            )
                    for n, (o, nt) in enumerate(zip(offs, tiles)):
                        nc.scalar.activation(
                            g_sb[:, f, o:o + nt], ps[:, n, :nt], AF.Gelu,
                            bias=b1_sb[:, f:f + 1],
                        )

                # layer 2: yT[dd,:] = sum_ff w2[ff,dd].T @ gT[ff,:]  (+ b2)
                for dd in range(ND):
                    w2_sb = w2p.tile([P, NF, P], dt.bfloat16, tag="w2")
                    nc.sync.dma_start(w2_sb[:], w2[dd])
                    ps2 = psum.tile([P, nslices, NT], dt.float32, tag="ps")
                    for ff in range(NF):
                        for n, (o, nt) in enumerate(zip(offs, tiles)):
                            nc.tensor.matmul(
                                ps2[:, n, :nt], w2_sb[:, ff, :],
                                g_sb[:, ff, o:o + nt],
                                start=(ff == 0), stop=(ff == NF - 1),
                            )
                    y_sb = yp.tile([P, CS], dt.float32, tag="y")
                    for n, (o, nt) in enumerate(zip(offs, tiles)):
                        nc.scalar.activation(
                            y_sb[:, o:o + nt], ps2[:, n, :nt], AF.Identity,
                            bias=b2_sb[:, dd:dd + 1],
                        )
                    nc.sync.dma_start(yt[:, dd, base:base + CS], y_sb[:])

    nc.compile()
    return nc


def _route(x_flat, router_w, router_b):
    """Replicate the reference router on host: softmax -> top-2 -> renorm."""
    logits = (x_flat @ router_w + router_b).astype(np.float64)
    logits -= logits.max(axis=-1, keepdims=True)
    probs = np.exp(logits)
    probs /= probs.sum(axis=-1, keepdims=True)
    # top-k with jax.lax.top_k tie-breaking (lower index wins)
    idx = np.argsort(-probs, axis=-1, kind="stable")[:, :TOPK]
    topw = np.take_along_axis(probs, idx, axis=-1)
    topw = topw / (topw.sum(axis=-1, keepdims=True) + 1e-8)
    return idx.astype(np.int32), topw.astype(np.float32)


def kernel(x, router_w, router_b, w1, b1, w2, b2, _trace=False, _result_box=None):
    from concourse.bass_utils import run_bass_kernel_spmd

    x = np.asarray(x, dtype=np.float32)
    x_flat = x.reshape(N, D)
    topk_idx, topk_w = _route(x_flat, np.asarray(router_w, np.float32),
                              np.asarray(router_b, np.float32))

    # token lists per expert
    tok_idx = []
    tok_w = []
    for e in range(E):
        t, k = np.nonzero(topk_idx == e)
        tok_idx.append(t.astype(np.int64))
        tok_w.append(topk_w[t, k])
    counts = [len(t) for t in tok_idx]
    C = max(128, -(-max(counts) // P) * P)

    key = C
    if key not in _cache:
        _cache[key] = _build(C)
    nc = _cache[key]

    w1 = np.asarray(w1)
    w2 = np.asarray(w2)
    in_maps = []
    for e in range(E):
        xe = np.zeros((C, D), np.float32)
        xe[:counts[e]] = x_flat[tok_idx[e]]
        xt = np.ascontiguousarray(
            xe.reshape(C, ND, P).transpose(2, 1, 0)).astype(BF16)
        w1h = np.ascontiguousarray(
            w1[e].reshape(ND, P, NF, P).transpose(2, 1, 0, 3)).astype(BF16)
        w2h = np.ascontiguousarray(
            w2[e].reshape(NF, P, ND, P).transpose(2, 1, 0, 3)).astype(BF16)
        b1h = np.ascontiguousarray(
            np.asarray(b1[e], np.float32).reshape(NF, P).T)
        b2h = np.ascontiguousarray(
            np.asarray(b2[e], np.float32).reshape(ND, P).T)
        in_maps.append({"xt": xt, "w1": w1h, "b1": b1h, "w2": w2h, "b2": b2h})

    res = run_bass_kernel_spmd(
        nc, in_maps, core_ids=list(range(E)),
        trace=_trace, trace_cores=list(range(E)) if _trace else None,
        stitch_traces=False,
    )
    if _result_box is not None:
        _result_box.append(res)

    out = x_flat.copy()
    for e in range(E):
        yt = res.results[e]["yt"]                      # [P, ND, C] f32
        y = yt.transpose(2, 1, 0).reshape(C, D)
        cnt = counts[e]
        if cnt:
            out[tok_idx[e]] += tok_w[e][:, None] * y[:cnt]
    return out.reshape(B, S, D)


# revision 8
# speedup vs baseline: 1.3095x; 1.2540x over previous
"""MoE kernel for Trainium2 (8 NeuronCores, expert-parallel sparse dispatch).

Problem (hardcoded): B=2, S=2048, D=1024, E=8 experts, F=4096, top-K=2.
out = x + sum_{k in top2} w_k * (gelu(x @ w1[e_k] + b1[e_k]) @ w2[e_k] + b2[e_k])

Strategy: the router (0.01% of FLOPs) runs on host; tokens are dispatched
expert-parallel to the 8 cores (core i gets expert i's routed tokens, padded
to capacity C). Each core runs a dense FFN over its C tokens in bf16 with
fp32 PSUM accumulation, everything in transposed layout ([dim, token]) so no
on-device transposes are needed. Host scatter-adds the weighted expert
outputs and the residual.
"""

import numpy as np
import ml_dtypes

B, S, D, E, F, TOPK = 2, 2048, 1024, 8, 4096, 2
N = B * S           # 4096 tokens
P = 128             # partitions
ND = D // P         # 8 chunks of the model dim
NF = F // P         # 32 chunks of the hidden dim
NT = 512            # token tile (matmul free dim; one PSUM bank of fp32)

BF16 = ml_dtypes.bfloat16

_cache = {}


def _tile_plan(C):
    """Split C tokens into matmul free-dim tiles (multiples of 128)."""
    tiles = [NT] * (C // NT)
    if C % NT:
        tiles.append(C % NT)
    return tiles


# Max tokens processed per weight-stream pass (PSUM: <=4 slices of 512,
# and SBUF must hold x + g for the whole super-tile).
SUPER = 1536


def _build(C):
    import concourse.mybir as mybir
    import concourse.tile as tile
    from concourse import bacc

    dt = mybir.dt
    AF = mybir.ActivationFunctionType

    nc = bacc.Bacc("TRN2", target_bir_lowering=False, debug=False)
    xt = nc.dram_tensor("xt", (P, ND, C), dt.bfloat16, kind="ExternalInput")
    w1 = nc.dram_tensor("w1", (NF, P, ND, P), dt.bfloat16, kind="ExternalInput")
    b1 = nc.dram_tensor("b1", (P, NF), dt.float32, kind="ExternalInput")
    w2 = nc.dram_tensor("w2", (ND, P, NF, P), dt.bfloat16, kind="ExternalInput")
    b2 = nc.dram_tensor("b2", (P, ND), dt.float32, kind="ExternalInput")
    yt = nc.dram_tensor("yt", (P, ND, C), dt.float32, kind="ExternalOutput")

    with tile.TileContext(nc) as tc:
        with (
            tc.tile_pool(name="consts", bufs=1) as consts,
            tc.tile_pool(name="xp", bufs=1) as xp,
            tc.tile_pool(name="w1p", bufs=4) as w1p,
            tc.tile_pool(name="w2p", bufs=2) as w2p,
            tc.tile_pool(name="gp", bufs=1) as gp,
            tc.tile_pool(name="yp", bufs=2) as yp,
            tc.tile_pool(name="psum", bufs=2, space="PSUM") as psum,
        ):
            b1_sb = consts.tile([P, NF], dt.float32)
            nc.sync.dma_start(b1_sb[:], b1[:])
            b2_sb = consts.tile([P, ND], dt.float32)
            nc.sync.dma_start(b2_sb[:], b2[:])

            for base in range(0, C, SUPER):
                CS = min(SUPER, C - base)
                tiles = _tile_plan(CS)
                nslices = len(tiles)
                offs = [sum(tiles[:i]) for i in range(nslices)]

                x_sb = xp.tile([P, ND, CS], dt.bfloat16, tag="x")
                for d in range(ND):
                    nc.sync.dma_start(x_sb[:, d, :], xt[:, d, base:base + CS])

                g_sb = gp.tile([P, NF, CS], dt.bfloat16, tag="g")
                # layer 1: hT[f,:] = sum_d w1[d,f].T @ xT[d,:]  -> gelu
                # One weight chunk feeds all token slices (LDW amortized),
                # PSUM holds the nslices accumulation banks per f.
                for f in range(NF):
                    w1_sb = w1p.tile([P, ND, P], dt.bfloat16, tag="w1")
                    nc.sync.dma_start(w1_sb[:], w1[f])
                    ps = psum.tile([P, nslices, NT], dt.float32, tag="ps")
                    for d in range(ND):
                        for n, (o, nt) in enumerate(zip(offs, tiles)):
                            nc.tensor.matmul(
                                ps[:, n, :nt], w1_sb[:, d, :],
                                x_sb[:, d, o:o + nt],
                                start=(d == 0), stop=(d == ND - 1),
                            )
                    for n, (o, nt) in enumerate(zip(offs, tiles)):
                        nc.scalar.activation(
                            g_sb[:, f, o:o + nt], ps[:, n, :nt], AF.Gelu,
                            bias=b1_sb[:, f:f + 1],
                        )

                # layer 2: yT[dd,:] = sum_ff w2[ff,dd].T @ gT[ff,:]  (+ b2)
                for dd in range(ND):
                    w2_sb = w2p.tile([P, NF, P], dt.bfloat16, tag="w2")
                    nc.sync.dma_start(w2_sb[:], w2[dd])
                    ps2 = psum.tile([P, nslices, NT], dt.float32, tag="ps")
                    for ff in range(NF):
                        for n, (o, nt) in enumerate(zip(offs, tiles)):
                            nc.tensor.matmul(
                                ps2[:, n, :nt], w2_sb[:, ff, :],
                                g_sb[:, ff, o:o + nt],
                                start=(ff == 0), stop=(ff == NF - 1),
                            )
                    y_sb = yp.tile([P, CS], dt.float32, tag="y")
                    for n, (o, nt) in enumerate(zip(offs, tiles)):
                        nc.scalar.activation(
                            y_sb[:, o:o + nt], ps2[:, n, :nt], AF.Identity,
                            bias=b2_sb[:, dd:dd + 1],
                        )
                    nc.sync.dma_start(yt[:, dd, base:base + CS], y_sb[:])

    nc.compile()
    return nc


def _route(x_flat, router_w, router_b):
    """Replicate the reference router on host: softmax -> top-2 -> renorm."""
    logits = (x_flat @ router_w + router_b).astype(np.float64)
    logits -= logits.max(axis=-1, keepdims=True)
    probs = np.exp(logits)
    probs /= probs.sum(axis=-1, keepdims=True)
    # top-k with jax.lax.top_k tie-breaking (lower index wins)
    idx = np.argsort(-probs, axis=-1, kind="stable")[:, :TOPK]
    topw = np.take_along_axis(probs, idx, axis=-1)
    topw = topw / (topw.sum(axis=-1, keepdims=True) + 1e-8)
    return idx.astype(np.int32), topw.astype(np.float32)


def kernel(x, router_w, router_b, w1, b1, w2, b2, _trace=False, _result_box=None):
    from concourse.bass_utils import run_bass_kernel_spmd

    x = np.asarray(x, dtype=np.float32)
    x_flat = x.reshape(N, D)
    topk_idx, topk_w = _route(x_flat, np.asarray(router_w, np.float32),
                              np.asarray(router_b, np.float32))

    # token lists per expert
    tok_idx = []
    tok_w = []
    for e in range(E):
        t, k = np.nonzero(topk_idx == e)
        tok_idx.append(t.astype(np.int64))
        tok_w.append(topk_w[t, k])
    counts = [len(t) for t in tok_idx]
    C = max(128, -(-max(counts) // 32) * 32)

    key = C
    if key not in _cache:
        _cache[key] = _build(C)
    nc = _cache[key]

    w1 = np.asarray(w1)
    w2 = np.asarray(w2)
    in_maps = []
    for e in range(E):
        xe = np.zeros((C, D), np.float32)
        xe[:counts[e]] = x_flat[tok_idx[e]]
        xt = np.ascontiguousarray(
            xe.reshape(C, ND, P).transpose(2, 1, 0)).astype(BF16)
        w1h = np.ascontiguousarray(
            w1[e].reshape(ND, P, NF, P).transpose(2, 1, 0, 3)).astype(BF16)
        w2h = np.ascontiguousarray(
            w2[e].reshape(NF, P, ND, P).transpose(2, 1, 0, 3)).astype(BF16)
        b1h = np.ascontiguousarray(
            np.asarray(b1[e], np.float32).reshape(NF, P).T)
        b2h = np.ascontiguousarray(
            np.asarray(b2[e], np.float32).reshape(ND, P).T)
        in_maps.append({"xt": xt, "w1": w1h, "b1": b1h, "w2": w2h, "b2": b2h})

    res = run_bass_kernel_spmd(
        nc, in_maps, core_ids=list(range(E)),
        trace=_trace, trace_cores=list(range(E)) if _trace else None,
        stitch_traces=False,
    )
    if _result_box is not None:
        _result_box.append(res)

    out = x_flat.copy()
    for e in range(E):
        yt = res.results[e]["yt"]                      # [P, ND, C] f32
        y = yt.transpose(2, 1, 0).reshape(C, D)
        cnt = counts[e]
        if cnt:
            out[tok_idx[e]] += tok_w[e][:, None] * y[:cnt]
    return out.reshape(B, S, D)
